# revision 1
# baseline (speedup 1.0000x reference)
"""AttentionPooling (segment_reduce) Trainium2 kernel.

att = sigmoid([input_rep, final_rep] @ W_lin.T + b_lin)
g   = att * (final_rep @ W_last.T + b_last)
out = segment_sum(g, graph_index, 16384)          # graph_index sorted

Strategy (8 NeuronCores, pure data-parallel, no collectives):
  graph_index is sorted, so a contiguous node range covers a contiguous
  graph range.  Host greedily packs whole graphs into "windows" of
  <= WIN_NODES nodes spanning <= 128 graphs; ~136 windows cover all 500k
  nodes = 8 cores x 17 windows.  Each core gets its windows as a padded
  node stream in feature-major bf16 layout (host pre-transposes + casts,
  so the device only does line-rate DMA and matmuls).  Per 128-node
  subtile the device does:
    ones[1,128].T @ biascat[1,512]      -> initializes PSUM with biases
                                           (K=1 matmuls row-packed via
                                           tile_position, groups of 4)
    xT_in.T   @ WlinT[:128]             -> att_pre cols   (accumulate)
    xT_fin0.T @ [WlinT[128:256]|WlastT[:128]]   (accumulate)
    xT_fin1.T @ [WlinT[256:]  |WlastT[128:]]    (accumulate)
    ACT: att = sigmoid(psum att cols)   -> bf16
    DVE: g = att * psum val cols        -> bf16
    DVE: onehot[n, j] = (iota[j] == local_graph_idx[n])      (bf16)
    PE : onehot.T @ g  += seg_psum[128 graphs, 256]   (whole window)
  After each window the [128, 256] f32 graph block is written out; host
  reassembles the window blocks into [16384, 256].
"""

import numpy as np
import ml_dtypes

import concourse.bacc as bacc
import concourse.tile as tile
from concourse import mybir
from concourse import bass_utils
from concourse._compat import with_exitstack

P = 128
HID = 256
WIN_SUB = 29                     # subtiles (128 nodes) per window
WIN_NODES = WIN_SUB * P          # 3712
WINDOWS_PER_CORE = 17
N_CORES = 8
NUM_GRAPHS = 16384
GMAX = P                         # graph span per window

BF16 = mybir.dt.bfloat16
F32 = mybir.dt.float32
npbf16 = ml_dtypes.bfloat16


# ----------------------------------------------------------------------------
# host-side planning
# ----------------------------------------------------------------------------

def _build_windows(gi: np.ndarray, num_graphs: int):
    """Greedy windows: contiguous whole-graph ranges, graph span <= GMAX,
    node count <= WIN_NODES.  Returns list of (gbase, gcnt, nstart, ncnt)."""
    counts = np.bincount(gi, minlength=num_graphs)
    starts = np.concatenate([[0], np.cumsum(counts)])
    wins = []
    g = 0
    while g < num_graphs:
        base = g
        nodes = 0
        cnt = 0
        while g < num_graphs and cnt < GMAX and nodes + counts[g] <= WIN_NODES:
            nodes += int(counts[g])
            cnt += 1
            g += 1
        if cnt == 0:
            raise ValueError(f"graph {g} has {counts[g]} nodes > {WIN_NODES}")
        wins.append((base, cnt, int(starts[base]), nodes))
    return wins


# ----------------------------------------------------------------------------
# device kernel
# ----------------------------------------------------------------------------

@with_exitstack
def _device_kernel(ctx, tc, out_ap, ins, n_windows):
    nc = tc.nc
    xin_ap, xf0_ap, xf1_ap, idx_ap, wlin0_ap, wcat0_ap, wcat1_ap, \
        biascat_ap, ones_ap, iota_ap = ins

    consts = ctx.enter_context(tc.tile_pool(name="consts", bufs=1))
    xpool = ctx.enter_context(tc.tile_pool(name="x", bufs=2))
    x0pool = ctx.enter_context(tc.tile_pool(name="x0", bufs=1))
    apool = ctx.enter_context(tc.tile_pool(name="act", bufs=4))
    ohpool = ctx.enter_context(tc.tile_pool(name="oh", bufs=6))
    outpool = ctx.enter_context(tc.tile_pool(name="out", bufs=2))
    ps_sub = ctx.enter_context(tc.tile_pool(name="ps_sub", bufs=7, space="PSUM"))
    ps_seg = ctx.enter_context(tc.tile_pool(name="ps_seg", bufs=1, space="PSUM"))

    # constants, loaded once; ordered so the first matmuls' inputs land first
    # (iota/idx are only needed by the DVE ~1us in; loaded after chunk 0)
    ones_t = consts.tile([P, P], BF16)
    nc.sync.dma_start(ones_t[:], ones_ap[:])
    biascat = consts.tile([P, 2 * HID], BF16)
    nc.sync.dma_start(biascat[:], biascat_ap[:])
    wlin0 = consts.tile([P, HID], BF16)
    nc.sync.dma_start(wlin0[:], wlin0_ap[:])
    wcat0 = consts.tile([P, 2 * HID], BF16)
    nc.sync.dma_start(wcat0[:], wcat0_ap[:])
    wcat1 = consts.tile([P, 2 * HID], BF16)
    nc.sync.dma_start(wcat1[:], wcat1_ap[:])
    iota_t = consts.tile([P, P], F32)
    idx_t = consts.tile([P, n_windows * WIN_SUB], F32)

    def emit_mms(ps, half, xin_t, xf0_t, xf1_t, col):
        """The 3 accumulating matmuls for one subtile into psum half.
        The short (256-col) xin matmul closes the group so the psum-complete
        signal that gates the sigmoid fires as early as possible."""
        o = 2 * HID * half
        nc.tensor.matmul(ps[:, o:o + 2 * HID], lhsT=xf0_t[:, col:col + P],
                         rhs=wcat0[:, :], start=False, stop=False)
        nc.tensor.matmul(ps[:, o:o + 2 * HID], lhsT=xf1_t[:, col:col + P],
                         rhs=wcat1[:, :], start=False, stop=False)
        nc.tensor.matmul(ps[:, o:o + HID], lhsT=xin_t[:, col:col + P],
                         rhs=wlin0[:, :], start=False, stop=True)

    def emit_bias(ps, half, row):
        # K=1 matmul, row-packed: up to 4 bias matmuls use distinct 32-row
        # groups (and distinct PSUM banks), so they run concurrently in the
        # array — one 512-col span initializes four subtile banks.
        o = 2 * HID * half
        nc.tensor.matmul(ps[:, o:o + 2 * HID],
                         lhsT=ones_t[32 * row:32 * row + 1, 0:P],
                         rhs=biascat[32 * row:32 * row + 1, :],
                         start=True, stop=False,
                         tile_position=(32 * row, 0))

    # subtiles per psum/bias group: 4 is the measured optimum; the window
    # remainder rides the last group (a lone 1-subtile group would pay a
    # full bias span + row-group transition by itself)
    GROUPS = [4] * (WIN_SUB // 4 - 1) + [4 + WIN_SUB % 4]
    GSTART = [sum(GROUPS[:i]) for i in range(len(GROUPS))]
    n_groups = len(GROUPS)
    for w in range(n_windows):
        base = w * WIN_NODES
        if w == 0:
            # first-window loads: per-group chunks for the first two groups
            # (so the first matmuls start after ~0.3 MB), one bulk DMA for
            # the remainder (DMA-issue instructions cost ~650 ns each)
            NCHUNK = 3
            group_tiles = []
            for q in range(NCHUNK):
                c0 = GSTART[q] * P
                cw = GROUPS[q] * P
                xt = x0pool.tile([P, cw], BF16, tag=f"x0i{q}")
                nc.sync.dma_start(xt[:], xin_ap[:, base + c0:base + c0 + cw])
                f0 = x0pool.tile([P, cw], BF16, tag=f"x0a{q}")
                nc.sync.dma_start(f0[:], xf0_ap[:, base + c0:base + c0 + cw])
                f1 = x0pool.tile([P, cw], BF16, tag=f"x0b{q}")
                nc.sync.dma_start(f1[:], xf1_ap[:, base + c0:base + c0 + cw])
                group_tiles.append((xt, f0, f1, c0))
                if q == 0:
                    nc.sync.dma_start(iota_t[:], iota_ap[:])
                    nc.sync.dma_start(idx_t[:], idx_ap[:])
            r0 = GSTART[NCHUNK] * P
            rw = WIN_NODES - r0
            xr = x0pool.tile([P, rw], BF16, tag="x0ir")
            nc.sync.dma_start(xr[:], xin_ap[:, base + r0:base + WIN_NODES])
            f0r = x0pool.tile([P, rw], BF16, tag="x0ar")
            nc.sync.dma_start(f0r[:], xf0_ap[:, base + r0:base + WIN_NODES])
            f1r = x0pool.tile([P, rw], BF16, tag="x0br")
            nc.sync.dma_start(f1r[:], xf1_ap[:, base + r0:base + WIN_NODES])
            for q in range(NCHUNK, n_groups):
                group_tiles.append((xr, f0r, f1r, r0))
        else:
            xin_t = xpool.tile([P, WIN_NODES], BF16, tag="xin")
            nc.sync.dma_start(xin_t[:], xin_ap[:, base:base + WIN_NODES])
            xf0_t = xpool.tile([P, WIN_NODES], BF16, tag="xf0")
            nc.sync.dma_start(xf0_t[:], xf0_ap[:, base:base + WIN_NODES])
            xf1_t = xpool.tile([P, WIN_NODES], BF16, tag="xf1")
            nc.sync.dma_start(xf1_t[:], xf1_ap[:, base:base + WIN_NODES])
            group_tiles = [(xin_t, xf0_t, xf1_t, 0)] * n_groups

        seg = ps_seg.tile([P, HID], F32)

        def emit_body(ps, s, xt, f0, f1, colbase, oh):
            emit_mms(ps, 0, xt, f0, f1, s * P - colbase)
            att = apool.tile([P, HID], BF16, tag="att")
            nc.scalar.activation(att[:], ps[:, 0:HID],
                                 mybir.ActivationFunctionType.Sigmoid)
            g_sb = apool.tile([P, HID], BF16, tag="g")
            nc.vector.tensor_tensor(g_sb[:], att[:], ps[:, HID:2 * HID],
                                    op=mybir.AluOpType.mult)
            nc.tensor.matmul(seg[:, :], lhsT=oh[:], rhs=g_sb[:],
                             start=(s == 0), stop=(s == WIN_SUB - 1))

        # groups of 4 subtiles: the four K=1 bias matmuls are emitted first
        # with distinct row groups so the scheduler can pack them in the array
        for q in range(n_groups):
            subs = list(range(GSTART[q], GSTART[q] + GROUPS[q]))
            xt, f0, f1, colbase = group_tiles[q]
            ps_g = []
            for r, s in enumerate(subs):
                ps = ps_sub.tile([P, 2 * HID], F32, tag="ps")   # 1 bank
                ps_g.append(ps)
                emit_bias(ps, 0, r % 4)
            # one-hots first: they depend only on resident constants, and the
            # DVE is strict FIFO — built early they never sit behind a
            # gate-multiply that is itself waiting on ACT
            oh_g = []
            for s in subs:
                t = w * WIN_SUB + s
                oh = ohpool.tile([P, P], BF16)
                nc.vector.tensor_scalar(oh[:], iota_t[:], idx_t[:, t:t + 1],
                                        None, op0=mybir.AluOpType.is_equal)
                oh_g.append(oh)
            for ps, s, oh in zip(ps_g, subs, oh_g):
                emit_body(ps, s, xt, f0, f1, colbase, oh)

        out_t = outpool.tile([P, HID], F32)
        nc.scalar.copy(out_t[:], seg[:, :])
        nc.sync.dma_start(out_ap[w * P:(w + 1) * P, :], out_t[:])


def build_module(n_windows=WINDOWS_PER_CORE):
    nc = bacc.Bacc("TRN2", debug=False, num_devices=N_CORES)
    nn = n_windows * WIN_NODES
    ins = [
        nc.dram_tensor("xin", [P, nn], BF16, kind="ExternalInput").ap(),
        nc.dram_tensor("xf0", [P, nn], BF16, kind="ExternalInput").ap(),
        nc.dram_tensor("xf1", [P, nn], BF16, kind="ExternalInput").ap(),
        nc.dram_tensor("idx", [P, nn // P], F32, kind="ExternalInput").ap(),
        nc.dram_tensor("wlin0", [P, HID], BF16, kind="ExternalInput").ap(),
        nc.dram_tensor("wcat0", [P, 2 * HID], BF16, kind="ExternalInput").ap(),
        nc.dram_tensor("wcat1", [P, 2 * HID], BF16, kind="ExternalInput").ap(),
        nc.dram_tensor("biascat", [P, 2 * HID], BF16, kind="ExternalInput").ap(),
        nc.dram_tensor("ones", [P, P], BF16, kind="ExternalInput").ap(),
        nc.dram_tensor("iota", [P, P], F32, kind="ExternalInput").ap(),
    ]
    out_ap = nc.dram_tensor("out", [n_windows * P, HID], F32,
                            kind="ExternalOutput").ap()
    with tile.TileContext(nc) as tc:
        _device_kernel(tc, out_ap, ins, n_windows)
    nc.compile()
    return nc


# ----------------------------------------------------------------------------
# host-side data prep
# ----------------------------------------------------------------------------

def _prep(inputs, n_windows):
    gi = np.asarray(inputs["graph_index"]).astype(np.int64)
    x_in = np.asarray(inputs["input_rep"], dtype=np.float32)
    x_fin = np.asarray(inputs["final_rep"], dtype=np.float32)
    W_lin = np.asarray(inputs["W_lin"], dtype=np.float32)
    b_lin = np.asarray(inputs["b_lin"], dtype=np.float32)
    W_last = np.asarray(inputs["W_last"], dtype=np.float32)
    b_last = np.asarray(inputs["b_last"], dtype=np.float32)

    if np.any(np.diff(gi) < 0):
        order = np.argsort(gi, kind="stable")
        gi = gi[order]
        x_in = x_in[order]
        x_fin = x_fin[order]

    wins = _build_windows(gi, NUM_GRAPHS)
    budget = N_CORES * n_windows
    assert len(wins) <= budget, f"{len(wins)} windows > budget {budget}"
    wins = wins + [(NUM_GRAPHS, 0, len(gi), 0)] * (budget - len(wins))

    x_in_b = x_in.astype(npbf16)
    x_fin_b = x_fin.astype(npbf16)

    WlinT = W_lin.T.astype(npbf16)    # [384, 256]
    WlastT = W_last.T.astype(npbf16)  # [256, 256]
    wlin0 = np.ascontiguousarray(WlinT[0:P])
    wcat0 = np.ascontiguousarray(
        np.concatenate([WlinT[P:2 * P], WlastT[0:P]], axis=1))
    wcat1 = np.ascontiguousarray(
        np.concatenate([WlinT[2 * P:3 * P], WlastT[P:2 * P]], axis=1))
    biascat = np.tile(np.concatenate([b_lin, b_last])[None, :],
                      (P, 1)).astype(npbf16)
    ones_t = np.ones((P, P), npbf16)
    iota_t = np.tile(np.arange(P, dtype=np.float32)[None, :], (P, 1))

    nn = n_windows * WIN_NODES
    in_maps = []
    for c in range(N_CORES):
        xin_p = np.zeros((P, nn), npbf16)
        xf0_p = np.zeros((P, nn), npbf16)
        xf1_p = np.zeros((P, nn), npbf16)
        idx_p = np.full((P, nn // P), -1.0, np.float32)
        for j in range(n_windows):
            gb, gc, ns, ncnt = wins[c * n_windows + j]
            if ncnt == 0:
                continue
            off = j * WIN_NODES
            xin_p[:, off:off + ncnt] = x_in_b[ns:ns + ncnt].T
            xf0_p[:, off:off + ncnt] = x_fin_b[ns:ns + ncnt, 0:P].T
            xf1_p[:, off:off + ncnt] = x_fin_b[ns:ns + ncnt, P:2 * P].T
            flat = np.full((WIN_NODES,), -1.0, np.float32)
            flat[0:ncnt] = (gi[ns:ns + ncnt] - gb).astype(np.float32)
            cols = slice(off // P, (off + WIN_NODES) // P)
            idx_p[:, cols] = flat.reshape(-1, P).T
        in_maps.append({
            "xin": xin_p, "xf0": xf0_p, "xf1": xf1_p, "idx": idx_p,
            "wlin0": wlin0, "wcat0": wcat0, "wcat1": wcat1,
            "biascat": biascat, "ones": ones_t, "iota": iota_t,
        })
    return wins, in_maps


def _assemble(wins, results, n_windows):
    out = np.zeros((NUM_GRAPHS, HID), np.float32)
    for c in range(N_CORES):
        res = results[c]["out"]
        for j in range(n_windows):
            gb, gc, _, _ = wins[c * n_windows + j]
            if gc == 0:
                continue
            out[gb:gb + gc] = res[j * P:j * P + gc]
    return out


# ----------------------------------------------------------------------------
# entry point
# ----------------------------------------------------------------------------

_CACHE = {}
LAST_RESULTS = None


def kernel(**inputs) -> np.ndarray:
    global LAST_RESULTS
    gi = np.asarray(inputs["graph_index"]).astype(np.int64)
    n_wins_needed = len(_build_windows(np.sort(gi), NUM_GRAPHS))
    n_windows = max(WINDOWS_PER_CORE, -(-n_wins_needed // N_CORES))
    if n_windows not in _CACHE:
        _CACHE[n_windows] = build_module(n_windows)
    nc = _CACHE[n_windows]
    wins, in_maps = _prep(inputs, n_windows)
    # a previously-wedged core can fail one run with
    # NRT_EXEC_UNIT_UNRECOVERABLE and reset itself; retry once
    try:
        res = bass_utils.run_bass_kernel_spmd(
            nc, in_maps, core_ids=list(range(N_CORES)))
    except Exception:
        res = bass_utils.run_bass_kernel_spmd(
            nc, in_maps, core_ids=list(range(N_CORES)))
    LAST_RESULTS = res
    return _assemble(wins, res.results, n_windows)

